# revision 3
# baseline (speedup 1.0000x reference)
"""AttentionWeightedAverage distributed Trainium2 kernel.

Reference computation (all f32):
    s     = wv @ v + wg @ h          # (512, 384) + (512, 1) broadcast
    t     = tanh(s)                  # (512, 384)
    z     = wh @ t                   # (384, 384)
    alpha = softmax(z, axis=-1)      # (384, 384)
    out[i, j, l] = v[j, l] * alpha[i, j]   # (384, 384, 384)

The output is 226 MB while inputs are ~2.5 MB, so the kernel is bound by
the HBM write bandwidth of the broadcast product. Sharding: every core
gets the full (small) weights and computes s/t redundantly; core m owns
rows i in [m*48, (m+1)*48) of z/alpha and writes that contiguous slice
of the output. No collectives.

Per-core SBUF layouts (P = 128 partitions):
    v3   (128, 1152): v3[p, c*384+l]  = v[c*128+p, l]     c in 0..2
    wvT3 (128, 1536): wvT3[p, k*512+e] = wv[e, k*128+p]   k in 0..2
    wgT3 (128, 2048): wgT3[p, k*512+e] = wg[e, k*128+p]   k in 0..3
    h3   (128, 4):    h3[p, k]         = h[k*128+p, 0]
    whT3 (128, 192):  whT3[p, k*48+i]  = wh[m*48+i, k*128+p]
"""

import numpy as np

import concourse.bacc as bacc
import concourse.bass as bass
import concourse.mybir as mybir
from concourse import masks
from concourse.bass_utils import run_bass_kernel_spmd
from concourse.tile import TileContext

F32 = mybir.dt.float32
AF = mybir.ActivationFunctionType

NCORES = 8
L = 384          # vfeat_len == vfeat_dim
E = 512          # embed dim
IPC = L // NCORES  # 48 output rows per core
P = 128
CJ = L // P      # 3 chunks over the j axis
KV = L // P      # 3 contraction chunks for wv@v
KE = E // P      # 4 contraction chunks over embed dim
IPB = 2          # output rows batched per store DMA
OUT_BUFS = 6     # in-flight output tiles


def _build_nc() -> bacc.Bacc:
    nc = bacc.Bacc()

    v3_d = nc.declare_dram_parameter("v3", [P, CJ * L], F32, isOutput=False)
    wvT3_d = nc.declare_dram_parameter("wvT3", [P, KV * E], F32, isOutput=False)
    wgT3_d = nc.declare_dram_parameter("wgT3", [P, KE * E], F32, isOutput=False)
    h3_d = nc.declare_dram_parameter("h3", [P, KE], F32, isOutput=False)
    whT3_d = nc.declare_dram_parameter("whT3", [P, KE * IPC], F32, isOutput=False)
    out_d = nc.declare_dram_parameter("out", [IPC, L, L], F32, isOutput=True)

    with TileContext(nc) as tc:
        with (
            tc.tile_pool(name="const", bufs=1) as cpool,
            tc.tile_pool(name="work", bufs=2) as wpool,
            tc.tile_pool(name="psum", bufs=2, space="PSUM") as ppool,
            tc.tile_pool(name="outp", bufs=OUT_BUFS) as opool,
        ):
            # ---- input loads (chunked so dependent matmuls can start early)
            h_sb = cpool.tile([P, KE], F32)
            nc.sync.dma_start(out=h_sb[:], in_=h3_d[:])
            wgT_sb = cpool.tile([P, KE * E], F32)
            for k in range(KE):
                nc.sync.dma_start(
                    out=wgT_sb[:, k * E : (k + 1) * E],
                    in_=wgT3_d[:, k * E : (k + 1) * E],
                )
            v_sb = cpool.tile([P, CJ * L], F32)
            for c in range(CJ):
                nc.sync.dma_start(
                    out=v_sb[:, c * L : (c + 1) * L],
                    in_=v3_d[:, c * L : (c + 1) * L],
                )
            wvT_sb = cpool.tile([P, KV * E], F32)
            for k in range(KV):
                nc.sync.dma_start(
                    out=wvT_sb[:, k * E : (k + 1) * E],
                    in_=wvT3_d[:, k * E : (k + 1) * E],
                )
            whT_sb = cpool.tile([P, KE * IPC], F32)
            nc.sync.dma_start(out=whT_sb[:], in_=whT3_d[:])

            ident = cpool.tile([IPC, IPC], F32)
            masks.make_identity(nc, ident[:])

            # ---- gh = wg @ h  -> gh_sb[p, mc] = gh[mc*128 + p]
            gh_ps = ppool.tile([P, KE], F32)
            for mc in range(KE):
                for k in range(KE):
                    nc.tensor.matmul(
                        gh_ps[:, mc : mc + 1],
                        lhsT=wgT_sb[:, k * E + mc * P : k * E + (mc + 1) * P],
                        rhs=h_sb[:, k : k + 1],
                        start=(k == 0),
                        stop=(k == KE - 1),
                    )
            gh_sb = wpool.tile([P, KE], F32)
            nc.vector.tensor_copy(gh_sb[:], gh_ps[:])

            # ---- t = tanh(wv @ v + gh)  -> t3[p, mc*384 + j] = t[mc*128+p, j]
            t3 = cpool.tile([P, KE * L], F32)
            for mc in range(KE):
                s_ps = ppool.tile([P, L], F32, tag="s_ps")
                for k in range(KV):
                    nc.tensor.matmul(
                        s_ps[:],
                        lhsT=wvT_sb[:, k * E + mc * P : k * E + (mc + 1) * P],
                        rhs=v_sb[:, k * L : (k + 1) * L],
                        start=(k == 0),
                        stop=(k == KV - 1),
                    )
                nc.scalar.activation(
                    t3[:, mc * L : (mc + 1) * L],
                    s_ps[:],
                    AF.Tanh,
                    bias=gh_sb[:, mc : mc + 1],
                    scale=1.0,
                )

            # ---- z rows for this core: z[i, j], i in 0..47
            z_ps = ppool.tile([IPC, L], F32)
            for k in range(KE):
                nc.tensor.matmul(
                    z_ps[:],
                    lhsT=whT_sb[:, k * IPC : (k + 1) * IPC],
                    rhs=t3[:, k * L : (k + 1) * L],
                    start=(k == 0),
                    stop=(k == KE - 1),
                )

            # ---- softmax over j
            nmax = wpool.tile([IPC, 1], F32)
            nc.vector.tensor_reduce(
                nmax[:], z_ps[:], axis=mybir.AxisListType.X,
                op=mybir.AluOpType.max, negate=True,
            )
            e_sb = wpool.tile([IPC, L], F32)
            nc.scalar.activation(e_sb[:], z_ps[:], AF.Exp, bias=nmax[:], scale=1.0)
            rsum = wpool.tile([IPC, 1], F32)
            nc.vector.tensor_reduce(
                rsum[:], e_sb[:], axis=mybir.AxisListType.X, op=mybir.AluOpType.add,
            )
            rinv = wpool.tile([IPC, 1], F32)
            nc.vector.reciprocal(rinv[:], rsum[:])
            alpha = wpool.tile([IPC, L], F32)
            nc.vector.tensor_scalar_mul(alpha[:], e_sb[:], rinv[:])

            # ---- alphaT[p, c*48 + i] = alpha[i, c*128 + p]
            alphaT = wpool.tile([P, CJ * IPC], F32)
            for c in range(CJ):
                at_ps = ppool.tile([P, IPC], F32, tag="at_ps")
                nc.tensor.transpose(
                    at_ps[:], alpha[:, c * P : (c + 1) * P], ident[:]
                )
                nc.vector.tensor_copy(alphaT[:, c * IPC : (c + 1) * IPC], at_ps[:])

            # ---- out[i, c*128+p, l] = v[c*128+p, l] * alpha[i, c*128+p]
            for ib in range(0, IPC, IPB):
                ot = opool.tile([P, IPB * CJ * L], F32, tag="ot")
                for t in range(IPB):
                    i = ib + t
                    for c in range(CJ):
                        dst = ot[:, (t * CJ + c) * L : (t * CJ + c + 1) * L]
                        src = v_sb[:, c * L : (c + 1) * L]
                        sc = alphaT[:, c * IPC + i : c * IPC + i + 1]
                        if c < 2:
                            nc.vector.tensor_scalar_mul(dst, src, sc)
                        else:
                            nc.scalar.mul(dst, src, sc)
                dram_ap = out_d[ib : ib + IPB].rearrange(
                    "t (c p) l -> p t c l", c=CJ, p=P
                )
                sb_ap = ot.rearrange("p (t c l) -> p t c l", t=IPB, c=CJ)
                nc.sync.dma_start(out=dram_ap, in_=sb_ap)

    nc.compile()
    return nc


def _prep_inputs(h, v, wh, wv, wg):
    """Host-side relayout into the per-core SBUF-friendly layouts."""
    h = np.ascontiguousarray(h, dtype=np.float32)
    v = np.ascontiguousarray(v, dtype=np.float32)
    wh = np.ascontiguousarray(wh, dtype=np.float32)
    wv = np.ascontiguousarray(wv, dtype=np.float32)
    wg = np.ascontiguousarray(wg, dtype=np.float32)

    v3 = np.ascontiguousarray(
        v.reshape(CJ, P, L).transpose(1, 0, 2).reshape(P, CJ * L)
    )
    wvT3 = np.ascontiguousarray(
        wv.T.reshape(KV, P, E).transpose(1, 0, 2).reshape(P, KV * E)
    )
    wgT3 = np.ascontiguousarray(
        wg.T.reshape(KE, P, E).transpose(1, 0, 2).reshape(P, KE * E)
    )
    h3 = np.ascontiguousarray(h.reshape(KE, P).T)

    in_maps = []
    for m in range(NCORES):
        whm = wh[m * IPC : (m + 1) * IPC]  # (48, 512)
        whT3 = np.ascontiguousarray(
            whm.T.reshape(KE, P, IPC).transpose(1, 0, 2).reshape(P, KE * IPC)
        )
        in_maps.append(
            {"v3": v3, "wvT3": wvT3, "wgT3": wgT3, "h3": h3, "whT3": whT3}
        )
    return in_maps


def _run(inputs: dict, trace: bool = False, **kw):
    nc = _build_nc()
    in_maps = _prep_inputs(**inputs)
    res = run_bass_kernel_spmd(
        nc, in_maps, core_ids=list(range(NCORES)), trace=trace, **kw
    )
    out = np.concatenate([r["out"] for r in res.results], axis=0)
    return out, res


def kernel(h, v, wh, wv, wg):
    out, _ = _run({"h": h, "v": v, "wh": wh, "wv": wv, "wg": wg})
    return out


# revision 4
# speedup vs baseline: 1.1869x; 1.1869x over previous
"""AttentionWeightedAverage distributed Trainium2 kernel.

Reference computation (all f32):
    s     = wv @ v + wg @ h          # (512, 384) + (512, 1) broadcast
    t     = tanh(s)                  # (512, 384)
    z     = wh @ t                   # (384, 384)
    alpha = softmax(z, axis=-1)      # (384, 384)
    out[i, j, l] = v[j, l] * alpha[i, j]   # (384, 384, 384)

The output is 226 MB while inputs are ~2.5 MB, so the kernel is bound by
the HBM write bandwidth of the broadcast product (~358 GB/s per core ->
~79 us for the 28.3 MB per-core slice). Sharding: every core gets the
full (small) weights and computes s/t redundantly; core m owns rows
i in [m*48, (m+1)*48) of z/alpha and writes that contiguous slice of
the output. No collectives.

The prologue (everything before alpha is ready) is latency-critical:
- matmul operands are bf16 so LDWEIGHTS uses the fast weight load
  (fp32 LDW of a 128x128 tile costs ~0.85 us; bf16 ~0.1 us). PSUM
  accumulation stays f32 and the softmax + broadcast stay f32.
- wg @ h is folded into the s accumulation as a rank-1 (K=1) matmul
  with a ones row instead of 16 tiny N=1 matmuls.
- softmax skips the max-subtraction: |z| <= ||wh_row||_1 * max|tanh|
  < ~25, far from f32 exp overflow, and softmax is shift-invariant.
  The exp's accum_out gives the row sums for free.

Per-core SBUF layouts (P = 128 partitions):
    v3    (128, 1152) f32 : v3[p, c*384+l]  = v[c*128+p, l]   c in 0..2
    v3b   (128, 1152) bf16: same layout (matmul rhs copy)
    wvT3  (128, 1536) bf16: wvT3[p, k*512+e] = wv[e, k*128+p] k in 0..2
    wgT3  (128, 2048) bf16: wgT3[p, k*512+e] = wg[e, k*128+p] k in 0..3
    h3    (128, 4)    bf16: h3[p, k]         = h[k*128+p, 0]
    whT3  (128, 192)  bf16: whT3[p, k*48+i]  = wh[m*48+i, k*128+p]
"""

import numpy as np

import concourse.bacc as bacc
import concourse.bass as bass
import concourse.mybir as mybir
from concourse import masks
from concourse.bass_utils import run_bass_kernel_spmd
from concourse.tile import TileContext

F32 = mybir.dt.float32
BF16 = mybir.dt.bfloat16
AF = mybir.ActivationFunctionType

NCORES = 8
L = 384          # vfeat_len == vfeat_dim
E = 512          # embed dim
IPC = L // NCORES  # 48 output rows per core
P = 128
CJ = L // P      # 3 chunks over the j axis
KV = L // P      # 3 contraction chunks for wv@v
KE = E // P      # 4 contraction chunks over embed dim
IPB = 2          # output rows batched per store DMA
OUT_BUFS = 6     # in-flight output tiles


def _build_nc() -> bacc.Bacc:
    nc = bacc.Bacc()

    v3_d = nc.declare_dram_parameter("v3", [P, CJ * L], F32, isOutput=False)
    v3b_d = nc.declare_dram_parameter("v3b", [P, CJ * L], BF16, isOutput=False)
    wvT3_d = nc.declare_dram_parameter("wvT3", [P, KV * E], BF16, isOutput=False)
    wgT3_d = nc.declare_dram_parameter("wgT3", [P, KE * E], BF16, isOutput=False)
    h3_d = nc.declare_dram_parameter("h3", [P, KE], BF16, isOutput=False)
    whT3_d = nc.declare_dram_parameter("whT3", [P, KE * IPC], BF16, isOutput=False)
    out_d = nc.declare_dram_parameter("out", [IPC, L, L], F32, isOutput=True)

    with TileContext(nc) as tc:
        with (
            tc.tile_pool(name="const", bufs=1) as cpool,
            tc.tile_pool(name="work", bufs=2) as wpool,
            tc.tile_pool(name="psum", bufs=2, space="PSUM") as ppool,
            tc.tile_pool(name="outp", bufs=OUT_BUFS) as opool,
        ):
            # ---- input loads; split across the two HWDGE queues (SP + ACT)
            h_sb = cpool.tile([P, KE], BF16)
            nc.scalar.dma_start(out=h_sb[:], in_=h3_d[:])
            wgT_sb = cpool.tile([P, KE * E], BF16)
            nc.scalar.dma_start(out=wgT_sb[:], in_=wgT3_d[:])
            wvT_sb = cpool.tile([P, KV * E], BF16)
            nc.sync.dma_start(out=wvT_sb[:], in_=wvT3_d[:])
            vb_sb = cpool.tile([P, CJ * L], BF16)
            nc.sync.dma_start(out=vb_sb[:], in_=v3b_d[:])
            whT_sb = cpool.tile([P, KE * IPC], BF16)
            nc.sync.dma_start(out=whT_sb[:], in_=whT3_d[:])
            v_sb = cpool.tile([P, CJ * L], F32)
            nc.sync.dma_start(out=v_sb[:], in_=v3_d[:])

            ones_sb = cpool.tile([1, L], BF16)
            nc.gpsimd.memset(ones_sb[:], 1.0)
            ident = cpool.tile([IPC, IPC], F32)
            masks.make_identity(nc, ident[:])

            # ---- ghT[0, e] = (wg @ h)[e], e in 0..511
            ghT_ps = ppool.tile([1, E], F32)
            for k in range(KE):
                nc.tensor.matmul(
                    ghT_ps[:],
                    lhsT=h_sb[:, k : k + 1],
                    rhs=wgT_sb[:, k * E : (k + 1) * E],
                    start=(k == 0),
                    stop=(k == KE - 1),
                )
            ghT_sb = wpool.tile([1, E], BF16)
            nc.vector.tensor_copy(ghT_sb[:], ghT_ps[:])

            # ---- t = tanh(wv @ v + gh . 1^T), t3[p, mc*384+j] = t[mc*128+p, j]
            t3 = cpool.tile([P, KE * L], BF16)
            for mc in range(KE):
                s_ps = ppool.tile([P, L], F32, tag="s_ps")
                nc.tensor.matmul(
                    s_ps[:],
                    lhsT=ghT_sb[:, mc * P : (mc + 1) * P],
                    rhs=ones_sb[:],
                    start=True,
                    stop=False,
                )
                for k in range(KV):
                    nc.tensor.matmul(
                        s_ps[:],
                        lhsT=wvT_sb[:, k * E + mc * P : k * E + (mc + 1) * P],
                        rhs=vb_sb[:, k * L : (k + 1) * L],
                        start=False,
                        stop=(k == KV - 1),
                    )
                nc.scalar.activation(t3[:, mc * L : (mc + 1) * L], s_ps[:], AF.Tanh)

            # ---- z rows for this core: z[i, j], i in 0..47
            z_ps = ppool.tile([IPC, L], F32)
            for k in range(KE):
                nc.tensor.matmul(
                    z_ps[:],
                    lhsT=whT_sb[:, k * IPC : (k + 1) * IPC],
                    rhs=t3[:, k * L : (k + 1) * L],
                    start=(k == 0),
                    stop=(k == KE - 1),
                )

            # ---- softmax over j (no max shift; fused row sums)
            e_sb = wpool.tile([IPC, L], F32)
            rsum = wpool.tile([IPC, 1], F32)
            nc.scalar.activation(e_sb[:], z_ps[:], AF.Exp, accum_out=rsum[:])
            rinv = wpool.tile([IPC, 1], F32)
            nc.vector.reciprocal(rinv[:], rsum[:])
            alpha = wpool.tile([IPC, L], F32)
            nc.vector.tensor_scalar_mul(alpha[:], e_sb[:], rinv[:])

            # ---- alphaT[p, c*48 + i] = alpha[i, c*128 + p]
            alphaT = wpool.tile([P, CJ * IPC], F32)
            for c in range(CJ):
                at_ps = ppool.tile([P, IPC], F32, tag="at_ps")
                nc.tensor.transpose(
                    at_ps[:], alpha[:, c * P : (c + 1) * P], ident[:]
                )
                nc.vector.tensor_copy(alphaT[:, c * IPC : (c + 1) * IPC], at_ps[:])

            # ---- out[i, c*128+p, l] = v[c*128+p, l] * alpha[i, c*128+p]
            for ib in range(0, IPC, IPB):
                ot = opool.tile([P, IPB * CJ * L], F32, tag="ot")
                for t in range(IPB):
                    i = ib + t
                    for c in range(CJ):
                        dst = ot[:, (t * CJ + c) * L : (t * CJ + c + 1) * L]
                        src = v_sb[:, c * L : (c + 1) * L]
                        sc = alphaT[:, c * IPC + i : c * IPC + i + 1]
                        if c < 2 or i % 2 == 0:
                            nc.vector.tensor_scalar_mul(dst, src, sc)
                        else:
                            nc.scalar.mul(dst, src, sc)
                dram_ap = out_d[ib : ib + IPB].rearrange(
                    "t (c p) l -> p t c l", c=CJ, p=P
                )
                sb_ap = ot.rearrange("p (t c l) -> p t c l", t=IPB, c=CJ)
                nc.sync.dma_start(out=dram_ap, in_=sb_ap)

    nc.compile()
    return nc


def _prep_inputs(h, v, wh, wv, wg):
    """Host-side relayout into the per-core SBUF-friendly layouts."""
    h = np.ascontiguousarray(h, dtype=np.float32)
    v = np.ascontiguousarray(v, dtype=np.float32)
    wh = np.ascontiguousarray(wh, dtype=np.float32)
    wv = np.ascontiguousarray(wv, dtype=np.float32)
    wg = np.ascontiguousarray(wg, dtype=np.float32)

    def bf16(x):
        import ml_dtypes

        return np.ascontiguousarray(x.astype(ml_dtypes.bfloat16))

    v3 = np.ascontiguousarray(
        v.reshape(CJ, P, L).transpose(1, 0, 2).reshape(P, CJ * L)
    )
    wvT3 = bf16(wv.T.reshape(KV, P, E).transpose(1, 0, 2).reshape(P, KV * E))
    wgT3 = bf16(wg.T.reshape(KE, P, E).transpose(1, 0, 2).reshape(P, KE * E))
    h3 = bf16(h.reshape(KE, P).T)

    in_maps = []
    for m in range(NCORES):
        whm = wh[m * IPC : (m + 1) * IPC]  # (48, 512)
        whT3 = bf16(
            whm.T.reshape(KE, P, IPC).transpose(1, 0, 2).reshape(P, KE * IPC)
        )
        in_maps.append(
            {
                "v3": v3,
                "v3b": bf16(v3),
                "wvT3": wvT3,
                "wgT3": wgT3,
                "h3": h3,
                "whT3": whT3,
            }
        )
    return in_maps


def _run(inputs: dict, trace: bool = False, **kw):
    nc = _build_nc()
    in_maps = _prep_inputs(**inputs)
    res = run_bass_kernel_spmd(
        nc, in_maps, core_ids=list(range(NCORES)), trace=trace, **kw
    )
    out = np.concatenate([r["out"] for r in res.results], axis=0)
    return out, res


def kernel(h, v, wh, wv, wg):
    out, _ = _run({"h": h, "v": v, "wh": wh, "wv": wv, "wg": wg})
    return out


# revision 5
# speedup vs baseline: 1.2002x; 1.0113x over previous
"""AttentionWeightedAverage distributed Trainium2 kernel.

Reference computation (all f32):
    s     = wv @ v + wg @ h          # (512, 384) + (512, 1) broadcast
    t     = tanh(s)                  # (512, 384)
    z     = wh @ t                   # (384, 384)
    alpha = softmax(z, axis=-1)      # (384, 384)
    out[i, j, l] = v[j, l] * alpha[i, j]   # (384, 384, 384)

The output is 226 MB while inputs are ~2.5 MB, so the kernel is bound by
the HBM write bandwidth of the broadcast product (~358 GB/s per core ->
~79 us for the 28.3 MB per-core slice). Sharding: every core gets the
full (small) weights and computes s/t redundantly; core m owns rows
i in [m*48, (m+1)*48) of z/alpha and writes that contiguous slice of
the output. No collectives.

The prologue (everything before alpha is ready) is latency-critical:
- matmul operands are bf16 so LDWEIGHTS uses the fast weight load
  (fp32 LDW of a 128x128 tile costs ~0.85 us; bf16 ~0.1 us). PSUM
  accumulation stays f32 and the softmax + broadcast stay f32.
- wg @ h is folded into the s accumulation as a rank-1 (K=1) matmul
  with a ones row instead of 16 tiny N=1 matmuls.
- softmax skips the max-subtraction: |z| <= ||wh_row||_1 * max|tanh|
  < ~25, far from f32 exp overflow, and softmax is shift-invariant.
  The exp's accum_out gives the row sums for free.

Per-core SBUF layouts (P = 128 partitions):
    v3    (128, 1152) f32 : v3[p, c*384+l]  = v[c*128+p, l]   c in 0..2
    v3b   (128, 1152) bf16: same layout (matmul rhs copy)
    wvT3  (128, 1536) bf16: wvT3[p, k*512+e] = wv[e, k*128+p] k in 0..2
    wgT3  (128, 2048) bf16: wgT3[p, k*512+e] = wg[e, k*128+p] k in 0..3
    h3    (128, 4)    bf16: h3[p, k]         = h[k*128+p, 0]
    whT3  (128, 192)  bf16: whT3[p, k*48+i]  = wh[m*48+i, k*128+p]
"""

import numpy as np

import concourse.bacc as bacc
import concourse.bass as bass
import concourse.mybir as mybir
from concourse import masks
from concourse.bass_utils import run_bass_kernel_spmd
from concourse.tile import TileContext

F32 = mybir.dt.float32
BF16 = mybir.dt.bfloat16
AF = mybir.ActivationFunctionType

NCORES = 8
L = 384          # vfeat_len == vfeat_dim
E = 512          # embed dim
IPC = L // NCORES  # 48 output rows per core
P = 128
CJ = L // P      # 3 chunks over the j axis
KV = L // P      # 3 contraction chunks for wv@v
KE = E // P      # 4 contraction chunks over embed dim
IPB = 2          # output rows batched per store DMA
OUT_BUFS = 6     # in-flight output tiles


def _build_nc() -> bacc.Bacc:
    nc = bacc.Bacc()

    v3_d = nc.declare_dram_parameter("v3", [P, CJ * L], F32, isOutput=False)
    v3b_d = nc.declare_dram_parameter("v3b", [P, CJ * L], BF16, isOutput=False)
    wvT3_d = nc.declare_dram_parameter("wvT3", [P, KV * E], BF16, isOutput=False)
    wgT3_d = nc.declare_dram_parameter("wgT3", [P, KE * E], BF16, isOutput=False)
    h3_d = nc.declare_dram_parameter("h3", [P, KE], BF16, isOutput=False)
    whT3_d = nc.declare_dram_parameter("whT3", [P, KE * IPC], BF16, isOutput=False)
    out_d = nc.declare_dram_parameter("out", [IPC, L, L], F32, isOutput=True)

    with TileContext(nc) as tc:
        with (
            tc.tile_pool(name="const", bufs=1) as cpool,
            tc.tile_pool(name="work", bufs=2) as wpool,
            tc.tile_pool(name="psum", bufs=2, space="PSUM") as ppool,
            tc.tile_pool(name="outp", bufs=OUT_BUFS) as opool,
        ):
            # ---- input loads; split across the two HWDGE queues (SP + ACT)
            # and chunked along K so dependent matmuls start per-chunk.
            h_sb = cpool.tile([P, KE], BF16)
            nc.scalar.dma_start(out=h_sb[:], in_=h3_d[:])
            wgT_sb = cpool.tile([P, KE * E], BF16)
            for k in range(KE):
                nc.scalar.dma_start(
                    out=wgT_sb[:, k * E : (k + 1) * E],
                    in_=wgT3_d[:, k * E : (k + 1) * E],
                )
            wvT_sb = cpool.tile([P, KV * E], BF16)
            vb_sb = cpool.tile([P, CJ * L], BF16)
            for k in range(KV):
                nc.sync.dma_start(
                    out=wvT_sb[:, k * E : (k + 1) * E],
                    in_=wvT3_d[:, k * E : (k + 1) * E],
                )
                nc.sync.dma_start(
                    out=vb_sb[:, k * L : (k + 1) * L],
                    in_=v3b_d[:, k * L : (k + 1) * L],
                )
            whT_sb = cpool.tile([P, KE * IPC], BF16)
            nc.sync.dma_start(out=whT_sb[:], in_=whT3_d[:])
            v_sb = cpool.tile([P, CJ * L], F32)
            nc.sync.dma_start(out=v_sb[:], in_=v3_d[:])

            ones_sb = cpool.tile([1, L], BF16)
            nc.gpsimd.memset(ones_sb[:], 1.0)
            ident = cpool.tile([IPC, IPC], F32)
            masks.make_identity(nc, ident[:])

            # ---- ghT[0, e] = (wg @ h)[e], e in 0..511
            ghT_ps = ppool.tile([1, E], F32)
            for k in range(KE):
                nc.tensor.matmul(
                    ghT_ps[:],
                    lhsT=h_sb[:, k : k + 1],
                    rhs=wgT_sb[:, k * E : (k + 1) * E],
                    start=(k == 0),
                    stop=(k == KE - 1),
                )
            ghT_sb = wpool.tile([1, E], BF16)
            nc.vector.tensor_copy(ghT_sb[:], ghT_ps[:])

            # ---- t = tanh(wv @ v + gh . 1^T), t3[p, mc*384+j] = t[mc*128+p, j]
            t3 = cpool.tile([P, KE * L], BF16)
            for mc in range(KE):
                s_ps = ppool.tile([P, L], F32, tag="s_ps")
                nc.tensor.matmul(
                    s_ps[:],
                    lhsT=ghT_sb[:, mc * P : (mc + 1) * P],
                    rhs=ones_sb[:],
                    start=True,
                    stop=False,
                )
                for k in range(KV):
                    nc.tensor.matmul(
                        s_ps[:],
                        lhsT=wvT_sb[:, k * E + mc * P : k * E + (mc + 1) * P],
                        rhs=vb_sb[:, k * L : (k + 1) * L],
                        start=False,
                        stop=(k == KV - 1),
                    )
                nc.scalar.activation(t3[:, mc * L : (mc + 1) * L], s_ps[:], AF.Tanh)

            # ---- z rows for this core: z[i, j], i in 0..47
            z_ps = ppool.tile([IPC, L], F32)
            for k in range(KE):
                nc.tensor.matmul(
                    z_ps[:],
                    lhsT=whT_sb[:, k * IPC : (k + 1) * IPC],
                    rhs=t3[:, k * L : (k + 1) * L],
                    start=(k == 0),
                    stop=(k == KE - 1),
                )

            # ---- softmax over j (no max shift; fused row sums)
            e_sb = wpool.tile([IPC, L], F32)
            rsum = wpool.tile([IPC, 1], F32)
            nc.scalar.activation(e_sb[:], z_ps[:], AF.Exp, accum_out=rsum[:])
            rinv = wpool.tile([IPC, 1], F32)
            nc.vector.reciprocal(rinv[:], rsum[:])
            alpha = wpool.tile([IPC, L], F32)
            nc.vector.tensor_scalar_mul(alpha[:], e_sb[:], rinv[:])

            # ---- alphaT[p, c*48 + i] = alpha[i, c*128 + p]
            alphaT = wpool.tile([P, CJ * IPC], F32)
            for c in range(CJ):
                at_ps = ppool.tile([P, IPC], F32, tag="at_ps")
                nc.tensor.transpose(
                    at_ps[:], alpha[:, c * P : (c + 1) * P], ident[:]
                )
                nc.vector.tensor_copy(alphaT[:, c * IPC : (c + 1) * IPC], at_ps[:])

            # ---- out[i, c*128+p, l] = v[c*128+p, l] * alpha[i, c*128+p]
            for ib in range(0, IPC, IPB):
                ot = opool.tile([P, IPB * CJ * L], F32, tag="ot")
                for t in range(IPB):
                    i = ib + t
                    for c in range(CJ):
                        dst = ot[:, (t * CJ + c) * L : (t * CJ + c + 1) * L]
                        src = v_sb[:, c * L : (c + 1) * L]
                        sc = alphaT[:, c * IPC + i : c * IPC + i + 1]
                        if c < 2 or i % 2 == 0:
                            nc.vector.tensor_scalar_mul(dst, src, sc)
                        else:
                            nc.scalar.mul(dst, src, sc)
                dram_ap = out_d[ib : ib + IPB].rearrange(
                    "t (c p) l -> p t c l", c=CJ, p=P
                )
                sb_ap = ot.rearrange("p (t c l) -> p t c l", t=IPB, c=CJ)
                nc.sync.dma_start(out=dram_ap, in_=sb_ap)

    nc.compile()
    return nc


def _prep_inputs(h, v, wh, wv, wg):
    """Host-side relayout into the per-core SBUF-friendly layouts."""
    h = np.ascontiguousarray(h, dtype=np.float32)
    v = np.ascontiguousarray(v, dtype=np.float32)
    wh = np.ascontiguousarray(wh, dtype=np.float32)
    wv = np.ascontiguousarray(wv, dtype=np.float32)
    wg = np.ascontiguousarray(wg, dtype=np.float32)

    def bf16(x):
        import ml_dtypes

        return np.ascontiguousarray(x.astype(ml_dtypes.bfloat16))

    v3 = np.ascontiguousarray(
        v.reshape(CJ, P, L).transpose(1, 0, 2).reshape(P, CJ * L)
    )
    wvT3 = bf16(wv.T.reshape(KV, P, E).transpose(1, 0, 2).reshape(P, KV * E))
    wgT3 = bf16(wg.T.reshape(KE, P, E).transpose(1, 0, 2).reshape(P, KE * E))
    h3 = bf16(h.reshape(KE, P).T)

    in_maps = []
    for m in range(NCORES):
        whm = wh[m * IPC : (m + 1) * IPC]  # (48, 512)
        whT3 = bf16(
            whm.T.reshape(KE, P, IPC).transpose(1, 0, 2).reshape(P, KE * IPC)
        )
        in_maps.append(
            {
                "v3": v3,
                "v3b": bf16(v3),
                "wvT3": wvT3,
                "wgT3": wgT3,
                "h3": h3,
                "whT3": whT3,
            }
        )
    return in_maps


def _run(inputs: dict, trace: bool = False, **kw):
    nc = _build_nc()
    in_maps = _prep_inputs(**inputs)
    res = run_bass_kernel_spmd(
        nc, in_maps, core_ids=list(range(NCORES)), trace=trace, **kw
    )
    out = np.concatenate([r["out"] for r in res.results], axis=0)
    return out, res


def kernel(h, v, wh, wv, wg):
    out, _ = _run({"h": h, "v": v, "wh": wh, "wv": wv, "wg": wg})
    return out
